# revision 10
# baseline (speedup 1.0000x reference)
"""Trainium2 Bass kernel for nn_AutoregressiveDecoder.

Computes, for B=131072 rows (data-parallel over 8 NeuronCores):
  3-step autoregressive decoder: per step s,
    inp = [seq_embed, state, onehot(perm[:,s])]  (D+12)
    h = gelu(inp @ W1 + b1); h = gelu(h @ W2 + b2); pred = h @ W3 + b3
    masked losses + scatter decoded values into state.
Returns (autoreg_loss, dec_f, dec_p) like the reference.

Key optimizations:
  - seq_embed @ W1[:1024] is step-invariant -> computed once (not 3x).
  - all matmuls run as float32r (full-rate on the PE, bit-identical to
    fp32 matmul results on TRN2 hardware).
  - activations kept feature-on-partition (transposed); per-row scalar
    math done in row-on-partition layout via cheap PE transposes of the
    [2, rows] prediction tensor.
"""

import os
import sys
import types

for _p in ("/opt/trn_rl_repo",):
    if _p not in sys.path and os.path.isdir(_p):
        sys.path.insert(0, _p)

import numpy as np

import concourse.bass as bass
import concourse.mybir as mybir
import concourse.tile as tile
from concourse import bacc
from concourse.bass_utils import run_bass_kernel_spmd
from concourse.masks import make_identity

F32 = mybir.dt.float32
F32R = mybir.dt.float32r
AT = mybir.ActivationFunctionType
ALU = mybir.AluOpType

B, D, H = 131072, 1024, 128
N_CORES = 8
R = B // N_CORES              # rows per core
TILE = 512                    # rows per matmul tile (moving N)
CH = 128                      # rows per chunk (partition grid)
ALL_PERMS = np.array([[0, 1, 2], [0, 2, 1], [1, 0, 2],
                      [1, 2, 0], [2, 0, 1], [2, 1, 0]], dtype=np.int64)
# stateoh_T row order: [f0,p0,f1,p1,f2,p2, d0,d1,d2, oh0,oh1,oh2]
# original W1 row offsets (within W1[1024:]):  f_j->3j, p_j->3j+1, d_j->3j+2, oh_j->9+j
W1SO_PERM = [0, 1, 3, 4, 6, 7, 2, 5, 8, 9, 10, 11]


def _install_axon_ntff_hook():
    """Make trace=True work under axon when antenv.axon_hooks is absent."""
    try:
        import antenv  # noqa: F401
        import antenv.axon_hooks  # noqa: F401
        return
    except ImportError:
        pass
    try:
        import antenv
        from trn_agent_boot.trn_boot import _ntff_profile_via_ctypes
        mod = types.ModuleType("antenv.axon_hooks")
        _h = [None]
        mod.set_axon_ntff_profile_hook = lambda h: _h.__setitem__(0, h)
        mod.get_axon_ntff_profile_hook = lambda: _h[0]
        sys.modules["antenv.axon_hooks"] = mod
        antenv.axon_hooks = mod
        so = "/opt/axon/libaxon_pjrt.so"
        if os.path.exists(so):
            mod.set_axon_ntff_profile_hook(_ntff_profile_via_ctypes(so))
        import concourse.bass_utils as bu
        bu.upload_artifacts = lambda tmpdir: "local://skipped"
    except Exception:
        pass


def build_nc(rows=R, group_tiles=8):
    """Build the per-core Bass program (SPMD; identical on all cores)."""
    n_tiles = rows // TILE
    n_groups = n_tiles // group_tiles
    n_chunks = rows // CH               # chunk grid columns
    cpg = group_tiles * (TILE // CH)    # chunks per group
    n_gs = n_groups * 3                 # group-steps (loss col pairs)

    nc = bacc.Bacc("TRN2", target_bir_lowering=False, debug=False)

    seqT = nc.declare_dram_parameter("seqT", [D, rows], F32R, isOutput=False)
    w1 = nc.declare_dram_parameter("w1", [D, H], F32R, isOutput=False)
    w1so = nc.declare_dram_parameter("w1so", [12, H], F32R, isOutput=False)
    w2 = nc.declare_dram_parameter("w2", [H, H], F32R, isOutput=False)
    w3 = nc.declare_dram_parameter("w3", [H, 2], F32R, isOutput=False)
    b1 = nc.declare_dram_parameter("b1", [H, 1], F32, isOutput=False)
    b2 = nc.declare_dram_parameter("b2", [H, 1], F32, isOutput=False)
    b3 = nc.declare_dram_parameter("b3", [2, 1], F32, isOutput=False)
    stat0 = nc.declare_dram_parameter("stat0", [n_tiles, 12, TILE], F32R, isOutput=False)
    stat1 = nc.declare_dram_parameter("stat1", [n_tiles, 6, TILE], F32R, isOutput=False)
    stat2 = nc.declare_dram_parameter("stat2", [n_tiles, 6, TILE], F32R, isOutput=False)
    # grids: [p, step, chunk]
    gtf = nc.declare_dram_parameter("gtf", [CH, 3, n_chunks], F32, isOutput=False)
    gtp = nc.declare_dram_parameter("gtp", [CH, 3, n_chunks], F32, isOutput=False)
    mgr = nc.declare_dram_parameter("mgr", [CH, 3, n_chunks], F32, isOutput=False)
    rix = nc.declare_dram_parameter("rix", [CH, 3, n_chunks], F32, isOutput=False)
    dec_out = nc.declare_dram_parameter("dec_out", [CH, 6, n_chunks], F32, isOutput=True)
    loss_out = nc.declare_dram_parameter("loss_out", [CH, 2 * n_gs], F32, isOutput=True)

    stats_in = [stat0, stat1, stat2]

    with tile.TileContext(nc) as tc:
        with (
            tc.tile_pool(name="singles", bufs=1) as singles,
            tc.tile_pool(name="seqin", bufs=3) as seqin,
            tc.tile_pool(name="basep", bufs=2) as basep,
            tc.tile_pool(name="acts", bufs=4) as acts,
            tc.tile_pool(name="preds", bufs=3) as preds,
            tc.tile_pool(name="stats", bufs=20) as stats,
            tc.tile_pool(name="stage", bufs=2) as stage,
            tc.tile_pool(name="smath", bufs=8) as smath,
            tc.tile_pool(name="ps_base", bufs=2, space="PSUM") as ps_base,
            tc.tile_pool(name="ps_h", bufs=2, space="PSUM") as ps_h,
            tc.tile_pool(name="ps_pred", bufs=1, space="PSUM") as ps_pred,
            tc.tile_pool(name="ps_pn", bufs=1, space="PSUM") as ps_pn,
            tc.tile_pool(name="ps_st", bufs=2, space="PSUM") as ps_st,
        ):
            # ---- resident constants ----
            w1_sb = singles.tile([128, 8, H], F32R, tag="w1")
            nc.sync.dma_start(out=w1_sb, in_=w1.rearrange("(k p) h -> p k h", p=128))
            w1so_sb = singles.tile([12, H], F32R, tag="w1so")
            nc.sync.dma_start(out=w1so_sb, in_=w1so[:, :])
            w2_sb = singles.tile([H, H], F32R, tag="w2")
            nc.sync.dma_start(out=w2_sb, in_=w2[:, :])
            w3_sb = singles.tile([H, 2], F32R, tag="w3")
            nc.sync.dma_start(out=w3_sb, in_=w3[:, :])
            b1_sb = singles.tile([H, 1], F32, tag="b1")
            nc.sync.dma_start(out=b1_sb, in_=b1[:, :])
            b2_sb = singles.tile([H, 1], F32, tag="b2")
            nc.sync.dma_start(out=b2_sb, in_=b2[:, :])
            b3_sb = singles.tile([2, 1], F32, tag="b3")
            nc.sync.dma_start(out=b3_sb, in_=b3[:, :])
            ident_f = singles.tile([128, 128], F32, tag="identf")
            make_identity(nc, ident_f)
            ident = singles.tile([128, 128], F32R, tag="ident")
            nc.vector.tensor_copy(ident, ident_f)

            gtf_sb = singles.tile([CH, 3, n_chunks], F32, tag="gtf")
            nc.sync.dma_start(out=gtf_sb, in_=gtf[:, :, :])
            gtp_sb = singles.tile([CH, 3, n_chunks], F32, tag="gtp")
            nc.sync.dma_start(out=gtp_sb, in_=gtp[:, :, :])
            mgr_sb = singles.tile([CH, 3, n_chunks], F32, tag="mgr")
            nc.sync.dma_start(out=mgr_sb, in_=mgr[:, :, :])
            rix_sb = singles.tile([CH, 3, n_chunks], F32, tag="rix")
            nc.sync.dma_start(out=rix_sb, in_=rix[:, :, :])

            decf = singles.tile([CH, 3, n_chunks], F32, tag="decf")
            decp = singles.tile([CH, 3, n_chunks], F32, tag="decp")
            nc.vector.memset(decf, 0.0)
            nc.vector.memset(decp, 0.0)
            loss_sb = singles.tile([CH, 2 * n_gs], F32, tag="loss")

            seq_view = seqT.rearrange("(k p) r -> p k r", p=128)

            for g in range(n_groups):
                g_t0 = g * group_tiles
                gsl = slice(g_t0 * TILE, (g_t0 + group_tiles) * TILE)
                gch = slice(g * cpg, (g + 1) * cpg)

                # ---- phase A: base = seqT.T-chunks @ W1 for this group ----
                base_g = basep.tile([128, group_tiles, TILE], F32R, tag="base")
                for tl in range(group_tiles):
                    t = g_t0 + tl
                    seq_t = seqin.tile([128, 8, TILE], F32R, tag="seq")
                    nc.sync.dma_start(
                        out=seq_t,
                        in_=seq_view[:, :, t * TILE:(t + 1) * TILE],
                    )
                    pb = ps_base.tile([128, TILE], F32, tag="pb")
                    for k in range(8):
                        nc.tensor.matmul(
                            pb[:, :], w1_sb[:, k, :], seq_t[:, k, :],
                            start=(k == 0), stop=(k == 7),
                        )
                    nc.vector.tensor_copy(base_g[:, tl, :], pb[:, :])

                # ---- per-group stateoh tiles (3 steps x group_tiles) ----
                st_tiles = []
                for s in range(3):
                    prow = []
                    for tl in range(group_tiles):
                        st = stats.tile([12, TILE], F32R, tag="st")
                        if s == 0:
                            nc.sync.dma_start(out=st, in_=stats_in[0][g_t0 + tl])
                        else:
                            nc.sync.dma_start(out=st[6:12, :],
                                              in_=stats_in[s][g_t0 + tl])
                        prow.append(st)
                    st_tiles.append(prow)

                for s in range(3):
                    pn = ps_pn.tile([128, 2 * cpg], F32R, tag="pn")
                    for tl in range(group_tiles):
                        st = st_tiles[s][tl]
                        ph = ps_h.tile([128, TILE], F32, tag="ph")
                        nc.tensor.matmul(ph[:, :], w1so_sb[:, :], st[:, :],
                                         start=True, stop=False)
                        nc.tensor.matmul(ph[:, :], ident[:, :], base_g[:, tl, :],
                                         start=False, stop=True)
                        h1 = acts.tile([128, TILE], F32R, tag="h1")
                        nc.scalar.activation(h1, ph[:, :], AT.Gelu, bias=b1_sb[:, 0:1])
                        ph2 = ps_h.tile([128, TILE], F32, tag="ph")
                        nc.tensor.matmul(ph2[:, :], w2_sb[:, :], h1[:, :],
                                         start=True, stop=True)
                        h2 = acts.tile([128, TILE], F32R, tag="h1")
                        nc.scalar.activation(h2, ph2[:, :], AT.Gelu, bias=b2_sb[:, 0:1])
                        pp = ps_pred.tile([2, TILE], F32, tag="pp")
                        nc.tensor.matmul(pp[:, :], w3_sb[:, :], h2[:, :],
                                         start=True, stop=True)
                        pred = preds.tile([2, TILE], F32R, tag="pred")
                        nc.scalar.activation(pred, pp[:, :], AT.Identity,
                                             bias=b3_sb[:, 0:1])
                        for q in range(TILE // CH):
                            cl = tl * (TILE // CH) + q  # chunk-local index
                            nc.tensor.transpose(
                                pn[:, 2 * cl:2 * cl + 2],
                                pred[:, q * CH:(q + 1) * CH],
                                ident[:2, :2],
                            )

                    # ---- batched per-row scalar math on [128, cpg] ----
                    pf = pn[:, 0:2 * cpg:2]
                    z = pn[:, 1:2 * cpg:2]
                    gf = gtf_sb[:, s, gch]
                    gp = gtp_sb[:, s, gch]
                    m = mgr_sb[:, s, gch]
                    ri = rix_sb[:, s, gch]

                    sig = smath.tile([CH, cpg], F32, tag="sig")
                    nc.scalar.activation(sig, z, AT.Sigmoid)
                    az = smath.tile([CH, cpg], F32, tag="az")
                    nc.scalar.activation(az, z, AT.Abs)
                    ez = smath.tile([CH, cpg], F32, tag="ez")
                    nc.scalar.activation(ez, az, AT.Exp, scale=-1.0)
                    sp = smath.tile([CH, cpg], F32, tag="sp")
                    nc.scalar.activation(sp, ez, AT.Ln, bias=1.0)
                    mz = smath.tile([CH, cpg], F32, tag="mz")
                    nc.scalar.activation(mz, z, AT.Relu)

                    zg = smath.tile([CH, cpg], F32, tag="zg")
                    nc.vector.tensor_mul(zg, z, gp)
                    pl0 = smath.tile([CH, cpg], F32, tag="pl0")
                    nc.vector.tensor_sub(pl0, mz, zg)
                    nc.vector.tensor_add(pl0, pl0, sp)
                    # pl masked + reduce -> loss col
                    plm = smath.tile([CH, cpg], F32, tag="plm")
                    col = 2 * (3 * g + s)
                    nc.vector.scalar_tensor_tensor(
                        plm, pl0, 1.0, m, op0=ALU.mult, op1=ALU.mult,
                        accum_out=loss_sb[:, col + 1:col + 2],
                    )
                    # fl = ((pf-gf)*m)^2, reduce -> loss col
                    dm = smath.tile([CH, cpg], F32, tag="dm")
                    nc.vector.tensor_sub(dm, pf, gf)
                    dmm = smath.tile([CH, cpg], F32, tag="dmm")
                    nc.vector.tensor_mul(dmm, dm, m)
                    fl = smath.tile([CH, cpg], F32, tag="fl")
                    nc.vector.scalar_tensor_tensor(
                        fl, dmm, 1.0, dmm, op0=ALU.mult, op1=ALU.mult,
                        accum_out=loss_sb[:, col:col + 1],
                    )
                    # act_f / act_p (copy_predicated needs integer masks)
                    mint = smath.tile([CH, cpg], mybir.dt.uint8, tag="mint")
                    nc.vector.tensor_scalar(mint, m, 0.5, None, op0=ALU.is_gt)
                    af = smath.tile([CH, cpg], F32, tag="af")
                    nc.vector.tensor_copy(af, gf)
                    nc.vector.copy_predicated(af, mint, pf)
                    ap_ = smath.tile([CH, cpg], F32, tag="ap")
                    nc.vector.tensor_copy(ap_, gp)
                    nc.vector.copy_predicated(ap_, mint, sig)
                    # scatter into dec accumulators
                    for j in range(3):
                        ohj = smath.tile([CH, cpg], mybir.dt.uint8, tag="ohj")
                        nc.vector.tensor_scalar(ohj, ri, float(j), None,
                                                op0=ALU.is_equal)
                        nc.vector.copy_predicated(decf[:, j, gch], ohj, af)
                        nc.vector.copy_predicated(decp[:, j, gch], ohj, ap_)

                    # ---- rebuild stateoh rows (32q+0..6) for next step ----
                    if s < 2:
                        stg = stage.tile([CH, cpg, 6], F32R, tag="stg")
                        for c, (buf, j) in enumerate(
                            [(decf, 0), (decp, 0), (decf, 1),
                             (decp, 1), (decf, 2), (decp, 2)]
                        ):
                            nc.vector.tensor_copy(stg[:, :, c], buf[:, j, gch])
                        for tl in range(group_tiles):
                            pst = ps_st.tile([6, TILE], F32R, tag="pst")
                            for qq in range(TILE // CH):
                                cl = tl * (TILE // CH) + qq
                                nc.tensor.transpose(
                                    pst[:, qq * CH:(qq + 1) * CH],
                                    stg[:, cl, :],
                                    ident[:, :],
                                )
                            nxt = st_tiles[s + 1][tl]
                            nc.vector.tensor_copy(nxt[0:6, :], pst[:, :])

            # ---- outputs ----
            nc.sync.dma_start(out=dec_out[:, 0:3, :], in_=decf)
            nc.sync.dma_start(out=dec_out[:, 3:6, :], in_=decp)
            nc.sync.dma_start(out=loss_out[:, :], in_=loss_sb)

    nc.compile()
    return nc


_NC_CACHE = {}


def _get_nc(rows=R, group_tiles=8):
    key = (rows, group_tiles)
    if key not in _NC_CACHE:
        _NC_CACHE[key] = build_nc(rows, group_tiles)
    return _NC_CACHE[key]


def host_prep(seq_embed, freq, pres, round_mask, perm_idx,
              W1, b1, W2, b2, W3, b3, rows=R, n_cores=N_CORES):
    """Build per-core input maps + host-side reduction constants."""
    Btot = seq_embed.shape[0]
    n_chunks = rows // CH
    perms = ALL_PERMS[perm_idx.astype(np.int64)]          # [B, 3]
    eye3 = np.eye(3, dtype=np.float32)
    f32 = np.float32

    oh = [eye3[perms[:, s]] for s in range(3)]            # [B,3] each
    gtf_s = [np.take_along_axis(freq, perms[:, s:s+1], 1)[:, 0] for s in range(3)]
    gtp_s = [np.take_along_axis(pres, perms[:, s:s+1], 1)[:, 0] for s in range(3)]
    m_s = [np.take_along_axis(round_mask, perms[:, s:s+1], 1)[:, 0] for s in range(3)]

    w1so = np.ascontiguousarray(W1[D:][W1SO_PERM]).astype(f32)   # [12, H]

    def pack_stat(rowsX):
        """[nr, rows] -> per-tile [n_tiles, nr, TILE]."""
        nr = rowsX.shape[0]
        return np.ascontiguousarray(
            rowsX.reshape(nr, rows // TILE, TILE).transpose(1, 0, 2)).astype(f32)

    def grid(arrs, sl):
        # [3][R] -> [p, s, ch]
        a = np.stack([x[sl] for x in arrs])                # [3, R]
        return np.ascontiguousarray(
            a.reshape(3, n_chunks, CH).transpose(2, 0, 1)).astype(f32)

    in_maps = []
    for c in range(n_cores):
        sl = slice(c * rows, (c + 1) * rows)
        seqT = np.ascontiguousarray(seq_embed[sl].T)       # [D, rows]
        # stat0: full 12 rows (dec rows zero); stat1/2: [d0..2, oh0..2] only
        st0 = pack_stat(np.concatenate(
            [np.zeros((9, rows), f32), oh[0][sl].T.astype(f32)]))
        st1 = pack_stat(np.concatenate(
            [oh[0][sl].T, oh[1][sl].T]).astype(f32))
        st2 = pack_stat(np.concatenate(
            [(oh[0][sl] + oh[1][sl]).T, oh[2][sl].T]).astype(f32))
        rix = [perms[:, s].astype(np.float32) for s in range(3)]
        in_maps.append({
            "seqT": seqT,
            "w1": np.ascontiguousarray(W1[:D]).astype(f32),
            "w1so": w1so,
            "w2": np.ascontiguousarray(W2).astype(f32),
            "w3": np.ascontiguousarray(W3).astype(f32),
            "b1": np.ascontiguousarray(b1.reshape(H, 1)).astype(f32),
            "b2": np.ascontiguousarray(b2.reshape(H, 1)).astype(f32),
            "b3": np.ascontiguousarray(b3.reshape(2, 1)).astype(f32),
            "stat0": st0, "stat1": st1, "stat2": st2,
            "gtf": grid(gtf_s, sl), "gtp": grid(gtp_s, sl),
            "mgr": grid(m_s, sl), "rix": grid(rix, sl),
        })
    n_masked = f32(np.asarray(round_mask, np.float64).sum())
    return in_maps, n_masked


def assemble(results, n_masked, rows=R):
    n_chunks = rows // CH
    loss_sum = 0.0
    dec_f = np.empty((len(results) * rows, 3), np.float32)
    dec_p = np.empty((len(results) * rows, 3), np.float32)
    for c, out in enumerate(results):
        sl = slice(c * rows, (c + 1) * rows)
        do = out["dec_out"]                                # [p, 6, ch]
        dec_f[sl] = do[:, 0:3, :].transpose(2, 0, 1).reshape(rows, 3)
        dec_p[sl] = do[:, 3:6, :].transpose(2, 0, 1).reshape(rows, 3)
        loss_sum += out["loss_out"].astype(np.float64).sum()
    loss = np.float32(np.float32(loss_sum) / (n_masked + np.float32(1e-8)))
    return loss, dec_f, dec_p


def run(inputs, trace=False, rows=R, group_tiles=8):
    _install_axon_ntff_hook()
    nc = _get_nc(rows, group_tiles)
    in_maps, n_masked = host_prep(
        inputs["seq_embed"], inputs["freq"], inputs["pres"],
        inputs["round_mask"], inputs["perm_idx"],
        inputs["W1"], inputs["b1"], inputs["W2"], inputs["b2"],
        inputs["W3"], inputs["b3"], rows=rows,
        n_cores=B // rows,
    )
    res = run_bass_kernel_spmd(nc, in_maps, list(range(len(in_maps))),
                               trace=trace)
    loss, dec_f, dec_p = assemble(res.results, n_masked, rows=rows)
    return (loss, dec_f, dec_p), res


def kernel(**inputs):
    (loss, dec_f, dec_p), _ = run(inputs, trace=False)
    return loss, dec_f, dec_p


# revision 13
# speedup vs baseline: 1.1184x; 1.1184x over previous
"""Trainium2 Bass kernel for nn_AutoregressiveDecoder.

Computes, for B=131072 rows (data-parallel over 8 NeuronCores):
  3-step autoregressive decoder: per step s,
    inp = [seq_embed, state, onehot(perm[:,s])]  (D+12)
    h = gelu(inp @ W1 + b1); h = gelu(h @ W2 + b2); pred = h @ W3 + b3
    masked losses + scatter decoded values into state.
Returns (autoreg_loss, dec_f, dec_p) like the reference.

Key optimizations:
  - seq_embed @ W1[:1024] is step-invariant -> computed once (not 3x).
  - all matmuls run as float32r (full-rate on the PE, bit-identical to
    fp32 matmul results on TRN2 hardware).
  - activations kept feature-on-partition (transposed); per-row scalar
    math done in row-on-partition layout via cheap PE transposes of the
    [2, rows] prediction tensor.
"""

import os
import sys
import types

for _p in ("/opt/trn_rl_repo",):
    if _p not in sys.path and os.path.isdir(_p):
        sys.path.insert(0, _p)

import ml_dtypes
import numpy as np

import concourse.bass as bass
import concourse.mybir as mybir
import concourse.tile as tile
from concourse import bacc
from concourse.bass_utils import run_bass_kernel_spmd
from concourse.masks import make_identity

F32 = mybir.dt.float32
F32R = mybir.dt.float32r
BF = mybir.dt.bfloat16
AT = mybir.ActivationFunctionType
ALU = mybir.AluOpType

B, D, H = 131072, 1024, 128
N_CORES = 8
R = B // N_CORES              # rows per core
TILE = 512                    # rows per matmul tile (moving N)
CH = 128                      # rows per chunk (partition grid)
ALL_PERMS = np.array([[0, 1, 2], [0, 2, 1], [1, 0, 2],
                      [1, 2, 0], [2, 0, 1], [2, 1, 0]], dtype=np.int64)
# stateoh_T row order: [f0,p0,f1,p1,f2,p2, d0,d1,d2, oh0,oh1,oh2]
# original W1 row offsets (within W1[1024:]):  f_j->3j, p_j->3j+1, d_j->3j+2, oh_j->9+j
W1SO_PERM = [0, 1, 3, 4, 6, 7, 2, 5, 8, 9, 10, 11]


def _install_axon_ntff_hook():
    """Make trace=True work under axon when antenv.axon_hooks is absent."""
    try:
        import antenv  # noqa: F401
        import antenv.axon_hooks  # noqa: F401
        return
    except ImportError:
        pass
    try:
        import antenv
        from trn_agent_boot.trn_boot import _ntff_profile_via_ctypes
        mod = types.ModuleType("antenv.axon_hooks")
        _h = [None]
        mod.set_axon_ntff_profile_hook = lambda h: _h.__setitem__(0, h)
        mod.get_axon_ntff_profile_hook = lambda: _h[0]
        sys.modules["antenv.axon_hooks"] = mod
        antenv.axon_hooks = mod
        so = "/opt/axon/libaxon_pjrt.so"
        if os.path.exists(so):
            mod.set_axon_ntff_profile_hook(_ntff_profile_via_ctypes(so))
        import concourse.bass_utils as bu
        bu.upload_artifacts = lambda tmpdir: "local://skipped"
    except Exception:
        pass


def build_nc(rows=R, group_tiles=8):
    """Build the per-core Bass program (SPMD; identical on all cores)."""
    n_tiles = rows // TILE
    n_groups = n_tiles // group_tiles
    n_chunks = rows // CH               # chunk grid columns
    cpg = group_tiles * (TILE // CH)    # chunks per group
    n_gs = n_groups * 3                 # group-steps (loss col pairs)

    nc = bacc.Bacc("TRN2", target_bir_lowering=False, debug=False)

    seqT = nc.declare_dram_parameter("seqT", [D, rows], F32R, isOutput=False)
    w1 = nc.declare_dram_parameter("w1", [D, H], F32R, isOutput=False)
    w1so = nc.declare_dram_parameter("w1so", [12, H], BF, isOutput=False)
    w2 = nc.declare_dram_parameter("w2", [H, H], F32R, isOutput=False)
    w3 = nc.declare_dram_parameter("w3", [H, 2], F32R, isOutput=False)
    b1 = nc.declare_dram_parameter("b1", [H, 1], F32, isOutput=False)
    b2 = nc.declare_dram_parameter("b2", [H, 1], F32, isOutput=False)
    b3bc = nc.declare_dram_parameter("b3bc", [CH, 2], F32, isOutput=False)
    stat0 = nc.declare_dram_parameter("stat0", [n_tiles, 12, TILE], BF, isOutput=False)
    stat1 = nc.declare_dram_parameter("stat1", [n_tiles, 6, TILE], BF, isOutput=False)
    stat2 = nc.declare_dram_parameter("stat2", [n_tiles, 6, TILE], BF, isOutput=False)
    # grids: [p, step, chunk]
    gtf = nc.declare_dram_parameter("gtf", [CH, 3, n_chunks], F32, isOutput=False)
    gtp = nc.declare_dram_parameter("gtp", [CH, 3, n_chunks], F32, isOutput=False)
    mgr = nc.declare_dram_parameter("mgr", [CH, 3, n_chunks], F32, isOutput=False)
    rix = nc.declare_dram_parameter("rix", [CH, 3, n_chunks], F32, isOutput=False)
    dec_out = nc.declare_dram_parameter("dec_out", [CH, 6, n_chunks], F32, isOutput=True)
    loss_out = nc.declare_dram_parameter("loss_out", [CH, 2 * n_gs], F32, isOutput=True)

    stats_in = [stat0, stat1, stat2]

    with tile.TileContext(nc) as tc:
        with (
            tc.tile_pool(name="singles", bufs=1) as singles,
            tc.tile_pool(name="seqin", bufs=3) as seqin,
            tc.tile_pool(name="basep", bufs=2) as basep,
            tc.tile_pool(name="acts", bufs=6) as acts,
            tc.tile_pool(name="stats", bufs=20) as stats,
            tc.tile_pool(name="stage", bufs=2) as stage,
            tc.tile_pool(name="smath", bufs=8) as smath,
            tc.tile_pool(name="ps_base", bufs=2, space="PSUM") as ps_base,
            tc.tile_pool(name="ps_h", bufs=4, space="PSUM") as ps_h,
            tc.tile_pool(name="ps_pn", bufs=1, space="PSUM") as ps_pn,
            tc.tile_pool(name="ps_st", bufs=1, space="PSUM") as ps_st,
        ):
            # ---- resident constants ----
            w1_sb = singles.tile([128, 8, H], F32R, tag="w1")
            nc.sync.dma_start(out=w1_sb, in_=w1.rearrange("(k p) h -> p k h", p=128))
            w1so_sb = singles.tile([12, H], BF, tag="w1so")
            nc.sync.dma_start(out=w1so_sb, in_=w1so[:, :])
            w2_sb = singles.tile([H, H], F32R, tag="w2")
            nc.sync.dma_start(out=w2_sb, in_=w2[:, :])
            w3_sb = singles.tile([H, 2], F32R, tag="w3")
            nc.sync.dma_start(out=w3_sb, in_=w3[:, :])
            b1_sb = singles.tile([H, 1], F32, tag="b1")
            nc.sync.dma_start(out=b1_sb, in_=b1[:, :])
            b2_sb = singles.tile([H, 1], F32, tag="b2")
            nc.sync.dma_start(out=b2_sb, in_=b2[:, :])
            b3_sb = singles.tile([CH, 2], F32, tag="b3")
            nc.sync.dma_start(out=b3_sb, in_=b3bc[:, :])
            ident_f = singles.tile([128, 128], F32, tag="identf")
            make_identity(nc, ident_f)
            ident = singles.tile([128, 128], F32R, tag="ident")
            nc.vector.tensor_copy(ident, ident_f)
            ident_bf = singles.tile([128, 128], BF, tag="identbf")
            nc.vector.tensor_copy(ident_bf, ident_f)

            gtf_sb = singles.tile([CH, 3, n_chunks], F32, tag="gtf")
            nc.sync.dma_start(out=gtf_sb, in_=gtf[:, :, :])
            gtp_sb = singles.tile([CH, 3, n_chunks], F32, tag="gtp")
            nc.sync.dma_start(out=gtp_sb, in_=gtp[:, :, :])
            mgr_sb = singles.tile([CH, 3, n_chunks], F32, tag="mgr")
            nc.sync.dma_start(out=mgr_sb, in_=mgr[:, :, :])
            rix_sb = singles.tile([CH, 3, n_chunks], F32, tag="rix")
            nc.sync.dma_start(out=rix_sb, in_=rix[:, :, :])

            decf = singles.tile([CH, 3, n_chunks], F32, tag="decf")
            decp = singles.tile([CH, 3, n_chunks], F32, tag="decp")
            nc.vector.memset(decf, 0.0)
            nc.vector.memset(decp, 0.0)
            loss_sb = singles.tile([CH, 2 * n_gs], F32, tag="loss")

            seq_view = seqT.rearrange("(k p) r -> p k r", p=128)

            for g in range(n_groups):
                g_t0 = g * group_tiles
                gsl = slice(g_t0 * TILE, (g_t0 + group_tiles) * TILE)
                gch = slice(g * cpg, (g + 1) * cpg)

                # ---- phase A: base = seqT.T-chunks @ W1 for this group ----
                base_g = basep.tile([128, group_tiles, TILE], F32R, tag="base")
                for tl in range(group_tiles):
                    t = g_t0 + tl
                    seq_t = seqin.tile([128, 8, TILE], F32R, tag="seq")
                    nc.sync.dma_start(
                        out=seq_t,
                        in_=seq_view[:, :, t * TILE:(t + 1) * TILE],
                    )
                    pb = ps_base.tile([128, TILE], F32, tag="pb")
                    for k in range(8):
                        nc.tensor.matmul(
                            pb[:, :], w1_sb[:, k, :], seq_t[:, k, :],
                            start=(k == 0), stop=(k == 7),
                        )
                    nc.vector.tensor_copy(base_g[:, tl, :], pb[:, :])

                # ---- per-group stateoh tiles (3 steps x group_tiles) ----
                st_tiles = []
                for s in range(3):
                    prow = []
                    for tl in range(group_tiles):
                        st = stats.tile([12, TILE], BF, tag="st")
                        if s == 0:
                            nc.sync.dma_start(out=st, in_=stats_in[0][g_t0 + tl])
                        else:
                            nc.sync.dma_start(out=st[6:12, :],
                                              in_=stats_in[s][g_t0 + tl])
                        prow.append(st)
                    st_tiles.append(prow)

                for s in range(3):
                    pn = ps_pn.tile([128, 2 * cpg], F32, tag="pn")
                    for tl in range(group_tiles):
                        st = st_tiles[s][tl]
                        ph = ps_h.tile([128, TILE], F32, tag="ph")
                        nc.tensor.matmul(ph[:, :], w1so_sb[:, :], st[:, :],
                                         start=True, stop=False)
                        nc.tensor.matmul(ph[:, :], ident[:, :], base_g[:, tl, :],
                                         start=False, stop=True)
                        h1 = acts.tile([128, TILE], F32R, tag="h1")
                        nc.scalar.activation(h1, ph[:, :], AT.Gelu, bias=b1_sb[:, 0:1])
                        ph2 = ps_h.tile([128, TILE], F32, tag="ph")
                        nc.tensor.matmul(ph2[:, :], w2_sb[:, :], h1[:, :],
                                         start=True, stop=True)
                        h2 = acts.tile([128, TILE], F32R, tag="h1")
                        nc.scalar.activation(h2, ph2[:, :], AT.Gelu, bias=b2_sb[:, 0:1])
                        for q in range(TILE // CH):
                            cl = tl * (TILE // CH) + q  # chunk-local index
                            nc.tensor.matmul(
                                pn[:, 2 * cl:2 * cl + 2],
                                h2[:, q * CH:(q + 1) * CH],
                                w3_sb[:, :],
                                start=True, stop=True)

                    # ---- batched per-row scalar math on [128, cpg] ----
                    pn_sb = smath.tile([CH, 2 * cpg], F32, tag="pnsb")
                    nc.vector.tensor_copy(pn_sb, pn[:, :])
                    pf = pn_sb[:, 0:2 * cpg:2]
                    z = pn_sb[:, 1:2 * cpg:2]
                    gf = gtf_sb[:, s, gch]
                    gp = gtp_sb[:, s, gch]
                    m = mgr_sb[:, s, gch]
                    ri = rix_sb[:, s, gch]

                    # bias-adjusted preds (b3 folded here)
                    pfa = smath.tile([CH, cpg], F32, tag="pfa")
                    nc.vector.tensor_scalar(pfa, pf, b3_sb[:, 0:1], None,
                                            op0=ALU.add)
                    za = smath.tile([CH, cpg], F32, tag="za")
                    nc.vector.tensor_scalar(za, z, b3_sb[:, 1:2], None,
                                            op0=ALU.add)
                    sig = smath.tile([CH, cpg], F32, tag="sig")
                    nc.scalar.activation(sig, za, AT.Sigmoid)
                    # softplus(-|z|) = -ln(max(sig, 1-sig))
                    t1 = smath.tile([CH, cpg], F32, tag="t1")
                    nc.vector.tensor_scalar(t1, sig, -1.0, 1.0,
                                            op0=ALU.mult, op1=ALU.add)
                    nc.vector.tensor_max(t1, t1, sig)
                    lnt = smath.tile([CH, cpg], F32, tag="lnt")
                    nc.scalar.activation(lnt, t1, AT.Ln)
                    mz = smath.tile([CH, cpg], F32, tag="mz")
                    nc.vector.tensor_scalar_max(mz, za, 0.0)
                    zg = smath.tile([CH, cpg], F32, tag="zg")
                    nc.vector.tensor_mul(zg, za, gp)
                    p0 = smath.tile([CH, cpg], F32, tag="p0")
                    nc.vector.tensor_sub(p0, mz, zg)
                    pl0 = smath.tile([CH, cpg], F32, tag="pl0")
                    nc.vector.scalar_tensor_tensor(
                        pl0, lnt, -1.0, p0, op0=ALU.mult, op1=ALU.add)
                    # pl masked + reduce -> loss col
                    plm = smath.tile([CH, cpg], F32, tag="plm")
                    col = 2 * (3 * g + s)
                    nc.vector.scalar_tensor_tensor(
                        plm, pl0, 1.0, m, op0=ALU.mult, op1=ALU.mult,
                        accum_out=loss_sb[:, col + 1:col + 2],
                    )
                    # fl = ((pf-gf)*m)^2, reduce -> loss col
                    dm = smath.tile([CH, cpg], F32, tag="dm")
                    nc.vector.tensor_sub(dm, pfa, gf)
                    dmm = smath.tile([CH, cpg], F32, tag="dmm")
                    nc.vector.tensor_mul(dmm, dm, m)
                    fl = smath.tile([CH, cpg], F32, tag="fl")
                    nc.vector.scalar_tensor_tensor(
                        fl, dmm, 1.0, dmm, op0=ALU.mult, op1=ALU.mult,
                        accum_out=loss_sb[:, col:col + 1],
                    )
                    # act_f / act_p (copy_predicated needs integer masks)
                    mint = smath.tile([CH, cpg], mybir.dt.uint8, tag="mint")
                    nc.vector.tensor_scalar(mint, m, 0.5, None, op0=ALU.is_gt)
                    af = smath.tile([CH, cpg], F32, tag="af")
                    nc.vector.tensor_copy(af, gf)
                    nc.vector.copy_predicated(af, mint, pfa)
                    ap_ = smath.tile([CH, cpg], F32, tag="ap")
                    nc.vector.tensor_copy(ap_, gp)
                    nc.vector.copy_predicated(ap_, mint, sig)
                    # scatter into dec accumulators
                    for j in range(3):
                        ohj = smath.tile([CH, cpg], mybir.dt.uint8, tag="ohj")
                        nc.vector.tensor_scalar(ohj, ri, float(j), None,
                                                op0=ALU.is_equal)
                        nc.vector.copy_predicated(decf[:, j, gch], ohj, af)
                        nc.vector.copy_predicated(decp[:, j, gch], ohj, ap_)

                    # ---- rebuild stateoh rows (32q+0..6) for next step ----
                    if s < 2:
                        stg = stage.tile([CH, cpg, 6], BF, tag="stg")
                        for c, (buf, j) in enumerate(
                            [(decf, 0), (decp, 0), (decf, 1),
                             (decp, 1), (decf, 2), (decp, 2)]
                        ):
                            nc.vector.tensor_copy(stg[:, :, c], buf[:, j, gch])
                        for tl in range(group_tiles):
                            pst = ps_st.tile([6, TILE], BF, tag="pst")
                            for qq in range(TILE // CH):
                                cl = tl * (TILE // CH) + qq
                                nc.tensor.transpose(
                                    pst[:, qq * CH:(qq + 1) * CH],
                                    stg[:, cl, :],
                                    ident_bf[:, :],
                                )
                            nxt = st_tiles[s + 1][tl]
                            nc.vector.tensor_copy(nxt[0:6, :], pst[:, :])

            # ---- outputs ----
            nc.sync.dma_start(out=dec_out[:, 0:3, :], in_=decf)
            nc.sync.dma_start(out=dec_out[:, 3:6, :], in_=decp)
            nc.sync.dma_start(out=loss_out[:, :], in_=loss_sb)

    nc.compile()
    return nc


_NC_CACHE = {}


def _get_nc(rows=R, group_tiles=8):
    key = (rows, group_tiles)
    if key not in _NC_CACHE:
        _NC_CACHE[key] = build_nc(rows, group_tiles)
    return _NC_CACHE[key]


def host_prep(seq_embed, freq, pres, round_mask, perm_idx,
              W1, b1, W2, b2, W3, b3, rows=R, n_cores=N_CORES):
    """Build per-core input maps + host-side reduction constants."""
    Btot = seq_embed.shape[0]
    n_chunks = rows // CH
    perms = ALL_PERMS[perm_idx.astype(np.int64)]          # [B, 3]
    eye3 = np.eye(3, dtype=np.float32)
    f32 = np.float32

    oh = [eye3[perms[:, s]] for s in range(3)]            # [B,3] each
    gtf_s = [np.take_along_axis(freq, perms[:, s:s+1], 1)[:, 0] for s in range(3)]
    gtp_s = [np.take_along_axis(pres, perms[:, s:s+1], 1)[:, 0] for s in range(3)]
    m_s = [np.take_along_axis(round_mask, perms[:, s:s+1], 1)[:, 0] for s in range(3)]

    bf16 = ml_dtypes.bfloat16
    w1so = np.ascontiguousarray(W1[D:][W1SO_PERM]).astype(bf16)  # [12, H]

    def pack_stat(rowsX):
        """[nr, rows] -> per-tile [n_tiles, nr, TILE] (bf16)."""
        nr = rowsX.shape[0]
        return np.ascontiguousarray(
            rowsX.reshape(nr, rows // TILE, TILE).transpose(1, 0, 2)).astype(bf16)

    def grid(arrs, sl):
        # [3][R] -> [p, s, ch]
        a = np.stack([x[sl] for x in arrs])                # [3, R]
        return np.ascontiguousarray(
            a.reshape(3, n_chunks, CH).transpose(2, 0, 1)).astype(f32)

    in_maps = []
    for c in range(n_cores):
        sl = slice(c * rows, (c + 1) * rows)
        seqT = np.ascontiguousarray(seq_embed[sl].T)       # [D, rows]
        # stat0: full 12 rows (dec rows zero); stat1/2: [d0..2, oh0..2] only
        st0 = pack_stat(np.concatenate(
            [np.zeros((9, rows), f32), oh[0][sl].T.astype(f32)]))
        st1 = pack_stat(np.concatenate(
            [oh[0][sl].T, oh[1][sl].T]).astype(f32))
        st2 = pack_stat(np.concatenate(
            [(oh[0][sl] + oh[1][sl]).T, oh[2][sl].T]).astype(f32))
        rix = [perms[:, s].astype(np.float32) for s in range(3)]
        in_maps.append({
            "seqT": seqT,
            "w1": np.ascontiguousarray(W1[:D]).astype(f32),
            "w1so": w1so,
            "w2": np.ascontiguousarray(W2).astype(f32),
            "w3": np.ascontiguousarray(W3).astype(f32),
            "b1": np.ascontiguousarray(b1.reshape(H, 1)).astype(f32),
            "b2": np.ascontiguousarray(b2.reshape(H, 1)).astype(f32),
            "b3bc": np.ascontiguousarray(
                np.broadcast_to(b3.reshape(1, 2), (CH, 2))).astype(f32),
            "stat0": st0, "stat1": st1, "stat2": st2,
            "gtf": grid(gtf_s, sl), "gtp": grid(gtp_s, sl),
            "mgr": grid(m_s, sl), "rix": grid(rix, sl),
        })
    n_masked = f32(np.asarray(round_mask, np.float64).sum())
    return in_maps, n_masked


def assemble(results, n_masked, rows=R):
    n_chunks = rows // CH
    loss_sum = 0.0
    dec_f = np.empty((len(results) * rows, 3), np.float32)
    dec_p = np.empty((len(results) * rows, 3), np.float32)
    for c, out in enumerate(results):
        sl = slice(c * rows, (c + 1) * rows)
        do = out["dec_out"]                                # [p, 6, ch]
        dec_f[sl] = do[:, 0:3, :].transpose(2, 0, 1).reshape(rows, 3)
        dec_p[sl] = do[:, 3:6, :].transpose(2, 0, 1).reshape(rows, 3)
        loss_sum += out["loss_out"].astype(np.float64).sum()
    loss = np.float32(np.float32(loss_sum) / (n_masked + np.float32(1e-8)))
    return loss, dec_f, dec_p


def run(inputs, trace=False, rows=R, group_tiles=8):
    _install_axon_ntff_hook()
    nc = _get_nc(rows, group_tiles)
    in_maps, n_masked = host_prep(
        inputs["seq_embed"], inputs["freq"], inputs["pres"],
        inputs["round_mask"], inputs["perm_idx"],
        inputs["W1"], inputs["b1"], inputs["W2"], inputs["b2"],
        inputs["W3"], inputs["b3"], rows=rows,
        n_cores=B // rows,
    )
    res = run_bass_kernel_spmd(nc, in_maps, list(range(len(in_maps))),
                               trace=trace)
    loss, dec_f, dec_p = assemble(res.results, n_masked, rows=rows)
    return (loss, dec_f, dec_p), res


def kernel(**inputs):
    (loss, dec_f, dec_p), _ = run(inputs, trace=False)
    return loss, dec_f, dec_p
